# revision 42
# baseline (speedup 1.0000x reference)
"""Trainium2 Bass kernel for relative-position attention (nn_Attention_14714557956326).

Full inputs:
  x       [4, 1024, 1024] f32
  Wq      [1024, 1024]    f32   (dim -> 16 heads * 64)
  Wkv     [1024, 2048]    f32   (k cols 0..1023, v cols 1024..2047)
  pos_emb [1025, 64]      f32
Output: [4, 16, 1024, 64] f32  (softmax(q k^T * s + rel_pos_bias) v, per head)

Sharding: 8 cores; core c handles batch c//2, heads 8*(c%2) .. +8.
Everything else is per-core local (no collectives).

v3 pipeline per (head h, row-tile t):
  ext  = q_t . Trev           (PE, psum; only the column window the skew reads)
  pos  = skew-DMA(ext)        (SP-issued diagonal DMA)
  S    = ID^T pos + q_t^T k   (PE, accumulated into one [128,1024] psum tile)
  p    = exp(S - C), z = sum  (Act, reads psum directly)
  p   *= 1/z                  (DVE)
  ptv  = transpose(p)         (DmaTranspose, issue alternating SP/Act)
  out_h= V^T ptv              (PE)
Head 0's ext/skew prep is emitted inside the projection phase so the
attention pipeline starts hot.
"""

import sys

sys.path.insert(0, "/opt/trn_rl_repo")

import numpy as np
from contextlib import ExitStack

import concourse.bass as bass
import concourse.bacc as bacc
import concourse.tile as tile
from concourse import mybir
from concourse.ap import AP
from concourse.bass_utils import run_bass_kernel_spmd

# ---------------- problem constants ----------------
B = 4
N_HEADS_TOT = 16
D = 64
DIM = 1024
SEQ = 1024
MAX_POS = 512
TABLE = 2 * MAX_POS + 1  # 1025
SCALE = D ** -0.5

NC = 8              # cores
NH = 8              # heads per core
HD = NH * D         # 512 projected cols per matrix per core
KC = DIM // 128     # 8 contraction chunks
IT = SEQ // 128     # 8 row tiles
RW = 1024           # on-chip table cols c=0..1023; c=1024 via W* projection
EXT = 2 * TABLE - 3  # 2047 ext width
C_SHIFT = 8.0       # exp(x - C) to keep fp16 P in range

F32 = mybir.dt.float32
F32R = mybir.dt.float32r
F16 = mybir.dt.float16

_cached = {}


def build_nc(seq=SEQ, nh=NH, bench_iters=1, ablate=()):
    ablate = set(ablate)
    """Build the per-core Bass program (SPMD: same program on all 8 cores)."""
    it = seq // 128
    hd = nh * D

    nc = bacc.Bacc(
        "TRN2",
        target_bir_lowering=False,
        debug=False,
        enable_asserts=False,
        num_devices=NC,
    )

    # DRAM inputs declared f32r so DMA loads land directly in matmul-ready tiles
    xT = nc.dram_tensor("xT", [DIM, seq], F32R, kind="ExternalInput")
    Wq = nc.dram_tensor("Wq", [DIM, hd], F32R, kind="ExternalInput")
    Wk = nc.dram_tensor("Wk", [DIM, hd], F32R, kind="ExternalInput")
    Wv = nc.dram_tensor("Wv", [DIM, hd + nh], F32R, kind="ExternalInput")
    TrevT = nc.dram_tensor("TrevT", [D, RW], F32R, kind="ExternalInput")
    ID128 = nc.dram_tensor("ID128", [128, 128], F16, kind="ExternalInput")
    out = nc.dram_tensor("out", [nh, seq, D], F32, kind="ExternalOutput")

    with tile.TileContext(nc) as tc, ExitStack() as ctx:
        if bench_iters > 1:
            ctx.enter_context(
                tc.For_i(
                    0, bench_iters, 1,
                    hint_engines=(
                        mybir.EngineType.PE,
                        mybir.EngineType.DVE,
                        mybir.EngineType.Activation,
                        mybir.EngineType.SP,
                        mybir.EngineType.Pool,
                    ),
                    name="bench",
                )
            )
        # ---------------- persistent pools ----------------
        const_pool = ctx.enter_context(tc.tile_pool(name="const", bufs=1))
        qk_pool = ctx.enter_context(tc.tile_pool(name="qk", bufs=1))
        v_pool = ctx.enter_context(tc.tile_pool(name="v", bufs=1))
        # early attention pools: alive during projection (head-0 prep overlap)
        ext_pool = ctx.enter_context(tc.tile_pool(name="extp", bufs=1))
        pos_pool = ctx.enter_context(tc.tile_pool(name="posp", bufs=1))
        # PSUM pools (shared by projections and attention: 4+3+1 = 8 banks)
        psum_s = ctx.enter_context(tc.tile_pool(name="psS", bufs=2, space="PSUM"))
        psum_qp = ctx.enter_context(tc.tile_pool(name="psQP", bufs=3, space="PSUM"))
        psum_o = ctx.enter_context(tc.tile_pool(name="psO", bufs=1, space="PSUM"))

        # pos table, duplicated rows 0-63 and 64-127 so odd heads can slice base 64
        trev_dup = const_pool.tile([128, RW], F32R, tag="trevdup")
        exp_bias = const_pool.tile([128, 1], F32, tag="expbias")
        id128 = const_pool.tile([128, 128], F16, tag="id128")
        nc.gpsimd.memset(exp_bias[:], -C_SHIFT)
        nc.gpsimd.dma_start(id128[:], ID128[:])
        nc.sync.dma_start(trev_dup[0:64, :], TrevT[:])
        nc.scalar.dma_start(trev_dup[64:128, :], TrevT[:])

        qT = [const_pool.tile([128, seq], F32R, name=f"qT{m}", tag=f"qT{m}") for m in range(nh // 2)]
        kT = [qk_pool.tile([128, seq], F32R, name=f"kT{m}", tag=f"kT{m}") for m in range(nh // 2)]
        V = [v_pool.tile([128, nh * D], F16, name=f"V{j}", tag=f"V{j}") for j in range(it)]
        VQ = [v_pool.tile([128, nh], F16, name=f"VQ{j}", tag=f"VQ{j}") for j in range(it)]

        poss = {}
        pts = {}

        # ext matmul column windows (trev cols cc in [A, B) are the only ones
        # the skew DMA for row-tile t can read; cc=1024 (T[0]) handled via VQ)
        def ext_window(t):
            base = seq - 1 - 128 * t
            lo = max(511, base - 127)
            hi = min(1535, base + 1023)
            A = (lo - 511) & ~3  # even width for fp32r matmul ISA rules
            return A, hi - 511 + 1  # trev col range [A, B)

        def emit_qp_pair(hp, t, ci):
            # heads (2hp, 2hp+1) share one ext tile [128, 2*EXT] and ONE skew DMA
            q_pair = qT[hp]
            hs = (2 * hp, 2 * hp + 1)

            A, Bc = ext_window(t)
            Bc = min(Bc, RW)
            ext = ext_pool.tile(
                [128, 2 * EXT], F16, tag="ext", name=f"ext{hp}_{t}", bufs=3
            )
            lw = max(0, 511 - (seq - 128 - 128 * t))
            rw_ = min(max(0, (seq - 1 - 128 * t) + seq - 1 - 1535), EXT - 1536)
            for hh, h in enumerate(hs):
                base = 64 * hh
                q_t = q_pair[base : base + 64, 128 * t : 128 * (t + 1)]
                t_h = trev_dup[base : base + 64, :]
                eo = hh * EXT
                c0 = A
                qi = 0
                while c0 < Bc:
                    c1 = min(c0 + 512, Bc)
                    ps_qp = psum_qp.tile(
                        [128, 512], F32, tag="QP", name=f"qp{h}_{t}_{qi}", bufs=3
                    )
                    nc.tensor.matmul(
                        ps_qp[:, : c1 - c0], q_t, t_h[:, c0:c1], start=True, stop=True
                    )
                    eng = (nc.vector, nc.vector, nc.scalar)[(2 * ci + 2 * hh + qi) % 3]
                    if eng is nc.vector:
                        eng.tensor_copy(ext[:, eo + 511 + c0 : eo + 511 + c1], ps_qp[:, : c1 - c0])
                    else:
                        eng.copy(ext[:, eo + 511 + c0 : eo + 511 + c1], ps_qp[:, : c1 - c0])
                    c0 = c1
                    qi += 1
                if ext_window(t)[1] > 1024:  # skew reads col 1535 (T[0])
                    nc.gpsimd.tensor_copy(ext[:, eo + 1535 : eo + 1536], VQ[t][:, h : h + 1])
                if lw > 0:
                    nc.gpsimd.tensor_copy(
                        ext[:, eo + 511 - lw : eo + 511],
                        ext[:, eo + 511 : eo + 512].to_broadcast([128, lw]),
                    )
                if rw_ > 0:
                    nc.gpsimd.tensor_copy(
                        ext[:, eo + 1536 : eo + 1536 + rw_],
                        ext[:, eo + 1535 : eo + 1536].to_broadcast([128, rw_]),
                    )

            exts[(hp, t)] = ext

        exts = {}

        def emit_skew_pair(hp, t):
            hs = (2 * hp, 2 * hp + 1)
            ext = exts.pop((hp, t))
            pos = pos_pool.tile(
                [128, 2 * seq], F16, tag="pos", name=f"pos{hp}_{t}", bufs=12
            )
            poss[(hs[0], t)] = pos[:, 0:seq]
            poss[(hs[1], t)] = pos[:, seq : 2 * seq]
            extf = ext[:]
            diag = AP(
                tensor=extf.tensor,
                offset=extf.offset + (seq - 1 - 128 * t),
                ap=[[2 * EXT - 1, 128], [EXT, 2], [1, seq]],
            )
            nc.sync.dma_start(pos[:], diag)

        # ---------------- projections (+ head-0 pos prep overlap) ----------------
        with tc.tile_pool(name="wres", bufs=2) as w_pool, \
             tc.tile_pool(name="xres", bufs=1) as x_pool:

            wcols = hd + nh

            def load_w(wt, kc, ei):
                wr = w_pool.tile([128, wcols], F32R, tag=f"w{kc}")
                src = wt[128 * kc : 128 * (kc + 1), :]
                if wt is not Wv:
                    wr_v = wr[:, 0:hd]
                else:
                    wr_v = wr[:]
                (nc.sync, nc.scalar, nc.gpsimd)[ei % 3].dma_start(wr_v, src)
                return wr

            # direct f32r loads (no staging copies); interleave x with wv so
            # the v-projection's per-kc operands arrive together
            x_r = []
            wv_chunks = []
            for kc in range(KC):
                xr = x_pool.tile([128, seq], F32R, tag=f"xr{kc}")
                (nc.sync, nc.scalar, nc.gpsimd)[kc % 3].dma_start(
                    xr[:], xT[128 * kc : 128 * (kc + 1), :]
                )
                x_r.append(xr)
                wv_chunks.append(load_w(Wv, kc, kc + 1))

            def emit_qkT(wname, w_chunks, m, ci):
                # one head-pair tile [128, seq] of q^T or k^T
                dest = qT[m] if wname == "q" else kT[m]
                for sh in range(seq // 512):
                    ps = psum_qp.tile([128, 512], F32, tag="QP", bufs=3)
                    for kc in range(KC):
                        nc.tensor.matmul(
                            ps[:],
                            w_chunks[kc][:, 128 * m : 128 * (m + 1)],
                            x_r[kc][:, 512 * sh : 512 * (sh + 1)],
                            start=(kc == 0),
                            stop=(kc == KC - 1),
                        )
                    dst = dest[:, 512 * sh : 512 * (sh + 1)]
                    if ci % 2 == 0:
                        nc.vector.tensor_copy(dst, ps[:])
                    else:
                        nc.scalar.copy(dst, ps[:])
                    ci += 1
                return ci

            # v: out [seq, hd] tiles -> V fp16; plus vq1024 [128, nh] (g=0 col)
            for jt in range(it):
                ps = psum_s.tile([128, seq], F32, tag="S", name=f"vp{jt}", bufs=2)
                for kc in range(KC):
                    nc.tensor.matmul(
                        ps[:, 0:hd],
                        x_r[kc][:, 128 * jt : 128 * (jt + 1)],
                        wv_chunks[kc][:, 0:hd],
                        start=(kc == 0),
                        stop=(kc == KC - 1),
                    )
                    nc.tensor.matmul(
                        ps[:, seq - nh : seq],
                        x_r[kc][:, 128 * jt : 128 * (jt + 1)],
                        wv_chunks[kc][:, hd : hd + nh],
                        start=(kc == 0),
                        stop=(kc == KC - 1),
                    )
                if jt % 2 == 0:
                    nc.vector.tensor_copy(V[jt][:], ps[:, 0:hd])
                else:
                    nc.scalar.copy(V[jt][:], ps[:, 0:hd])
                nc.scalar.copy(VQ[jt][:], ps[:, seq - nh : seq])

            wq_chunks = [load_w(Wq, kc, kc + 1) for kc in range(KC)]
            ci = emit_qkT("q", wq_chunks, 0, 0)
            wk_chunks = [load_w(Wk, kc, kc) for kc in range(KC)]
            ci = emit_qkT("k", wk_chunks, 0, ci)

            # heads 0-1 pos prep overlaps the rest of projection
            cqp = 0
            for t in range(it):
                emit_qp_pair(0, t, cqp); cqp += 1
            for t in range(it):
                emit_skew_pair(0, t)
            for m in range(1, nh // 2):
                ci = emit_qkT("q", wq_chunks, m, ci)
            for m in range(1, nh // 2):
                ci = emit_qkT("k", wk_chunks, m, ci)

        # ---------------- attention ----------------
        att = ctx.enter_context(tc.tile_pool(name="att", bufs=3))
        pt_pool = ctx.enter_context(tc.tile_pool(name="pt", bufs=1))
        ot_pool = ctx.enter_context(tc.tile_pool(name="ot", bufs=3))

        ptalls, ptvs = {}, {}

        tr_engs = (nc.sync, nc.sync)  # transpose issue queues

        def emit_tile(h, t):
            pair, base = h // 2, 64 * (h % 2)
            q_h = qT[pair][base : base + 64, :]
            k_h = kT[pair][base : base + 64, :]
            q_t = q_h[:, 128 * t : 128 * (t + 1)]
            pos = poss.pop((h, t), None)

            ps_s = psum_s.tile([128, seq], F32, tag="S", name=f"s{h}_{t}", bufs=2)
            for jh in range(seq // 512):
                sl = slice(512 * jh, 512 * (jh + 1))
                if "add" not in ablate and pos is not None:
                    nc.tensor.matmul(
                        ps_s[:, sl], id128[:], pos[:, sl], start=True, stop=False
                    )
                    nc.tensor.matmul(ps_s[:, sl], q_t, k_h[:, sl], start=False, stop=True)
                else:
                    nc.tensor.matmul(ps_s[:, sl], q_t, k_h[:, sl], start=True, stop=True)

            if t % 4 == 0:
                pts["quad"] = att.tile(
                    [128, 4 * seq], F16, tag="pt", name=f"p{h}_{t}", bufs=2
                )
            p_pair = pts["quad"]
            p_t = p_pair[:, seq * (t % 4) : seq * (t % 4 + 1)]
            z_t = att.tile([128, 1], F32, tag="zt", name=f"z{h}_{t}", bufs=10)
            if "exp" not in ablate:
                nc.scalar.activation(
                    p_t, ps_s[:], mybir.ActivationFunctionType.Exp,
                    bias=exp_bias[:], scale=1.0, accum_out=z_t[:],
                )
            else:
                nc.vector.tensor_copy(p_t, ps_s[:])
                nc.gpsimd.memset(z_t[:], 1.0)
            pts[(h, t)] = (p_pair, z_t)

        quads = {}

        def emit_norm(h, t, ti):
            p_quad, z_t = pts.pop((h, t))
            p_t = p_quad[:, seq * (t % 4) : seq * (t % 4 + 1)]
            zr_t = att.tile([128, 1], F32, tag="zrt", name=f"zr{h}_{t}")
            nc.vector.reciprocal(zr_t[:], z_t[:])
            nc.vector.tensor_scalar_mul(p_t, p_t, zr_t[:])
            if t % 4 == 3:
                quads[(h, t // 4)] = p_quad

        def emit_transpose(h, Q, ti):
            # one quad transpose covers row-tiles 4Q..4Q+3; blocks land
            # f-major: 32 blocks of 128 at quad-group offset 4096*Q
            p_quad = quads.pop((h, Q))
            dst = ptalls[h][:, 4 * seq * Q : 4 * seq * (Q + 1)].rearrange(
                "p (b c) -> p b c", b=4 * it
            )
            teng = tr_engs[ti % 2]
            teng.dma_start_transpose(out=dst, in_=p_quad[:])

        ots = {}

        def emit_pv_part(h, pc):
            # i-chunk pair (2pc, 2pc+1); out rows i = 128*ch + p, layout [i, d].
            # ptall layout per quad-group Q: x = 4096*Q + qp*1024 + jb*128 + c
            # where i = 512*Q + 128*qp + c, j = 128*jb + partition
            ptv5 = ptalls[h].rearrange(
                "p (Q qp jb c) -> p Q qp jb c", Q=it // 4, qp=4, c=128
            )
            if pc == 0:
                ots[h] = ot_pool.tile([128, it * D], F32, tag="ot", name=f"ot{h}", bufs=3)
            ot = ots[h]
            ps_o = psum_o.tile([128, 128], F32, tag="O", name=f"o{h}_{pc}")
            for ch in (2 * pc, 2 * pc + 1):
                reg = ps_o[:, 64 * (ch % 2) : 64 * (ch % 2) + 64]
                for jb in range(it):
                    nc.tensor.matmul(
                        reg,
                        ptv5[:, ch // 4, ch % 4, jb, :],
                        V[jb][:, D * h : D * (h + 1)],
                        start=(jb == 0),
                        stop=(jb == it - 1),
                    )
            nc.vector.tensor_copy(ot[:, 128 * pc : 128 * (pc + 1)], ps_o[:])
            if pc == it // 2 - 1:
                dstv = out[h].rearrange("(c p) d -> p c d", c=it)
                (nc.sync, nc.scalar)[h % 2].dma_start(dstv, ots.pop(h)[:])

        def emit_pv(h):
            for pc in range(it // 2):
                emit_pv_part(h, pc)

        cqp = 0
        ti = 0
        for h in range(nh):
            ptalls[h] = pt_pool.tile(
                [128, it * seq], F16, tag=f"ptall{h % 2}", name=f"ptall{h}"
            )
        look = nh > 2
        for h in range(nh):
            for t in range(it):
                ahead = look and h % 2 == 0 and h + 2 < nh
                if ahead:
                    emit_qp_pair(h // 2 + 1, t, cqp); cqp += 1
                emit_tile(h, t)
                emit_norm(h, t, ti); ti += 1
                if ahead and t >= 2:
                    emit_skew_pair(h // 2 + 1, t - 2)
                if t == 4:
                    emit_transpose(h, 0, ti)
                if h > 0 and t % 2 == 1:
                    emit_pv_part(h - 1, t // 2)
                if h == nh - 1 and t == 5:
                    # last head: drain first-half PV once quad 0 transposed
                    emit_pv_part(h, 0)
                    emit_pv_part(h, 1)
            if look and h % 2 == 0 and h + 2 < nh:
                emit_skew_pair(h // 2 + 1, it - 2)
                emit_skew_pair(h // 2 + 1, it - 1)
            emit_transpose(h, 1, ti)
        emit_pv_part(nh - 1, 2)
        emit_pv_part(nh - 1, 3)

    nc.compile()
    return nc


def prep_inputs(x, Wq, Wkv, pos_emb):
    """Host-side shard prep: returns list of 8 per-core input dicts."""
    x = np.asarray(x, dtype=np.float32)
    Wq = np.asarray(Wq, dtype=np.float32)
    Wkv = np.asarray(Wkv, dtype=np.float32)
    pos_emb = np.asarray(pos_emb, dtype=np.float32)

    Wq_s = (Wq * SCALE).astype(np.float32)
    trevT = np.ascontiguousarray(pos_emb[::-1].T[:, :RW])    # [64, 1024], col c = T[1024-c]
    id128 = np.eye(128, dtype=np.float16)

    in_maps = []
    for c in range(NC):
        b, hg = c // 2, c % 2
        hs = slice(HD * hg, HD * (hg + 1))
        wq_c = np.ascontiguousarray(Wq_s[:, hs])
        # W*: per-head derived column so that x @ W*_h = q_h . T[0] (dist g=0)
        wstar = np.einsum(
            "dhe,e->dh", wq_c.reshape(DIM, NH, D), pos_emb[0].astype(np.float32)
        )
        in_maps.append(
            {
                "xT": np.ascontiguousarray(x[b].T),
                "Wq": wq_c,
                "Wk": np.ascontiguousarray(Wkv[:, hs]),
                "Wv": np.ascontiguousarray(
                    np.concatenate([Wkv[:, DIM:][:, hs], wstar], axis=1)
                ),
                "TrevT": trevT,
                "ID128": id128,
            }
        )
    return in_maps


def assemble(results):
    """results: list of 8 out maps (each {'out': [8, 1024, 64]}) -> [4,16,1024,64]."""
    out = np.empty((B, N_HEADS_TOT, SEQ, D), dtype=np.float32)
    for c in range(NC):
        b, hg = c // 2, c % 2
        out[b, NH * hg : NH * (hg + 1)] = results[c]["out"]
    return out


def kernel(x, Wq, Wkv, pos_emb, trace=False, trace_kwargs=None, bench_iters=1, ablate=()):
    key = ("nc", bench_iters, tuple(sorted(ablate)))
    if key not in _cached:
        _cached[key] = build_nc(bench_iters=bench_iters, ablate=ablate)
    nc = _cached[key]
    in_maps = prep_inputs(x, Wq, Wkv, pos_emb)
    res = run_bass_kernel_spmd(
        nc, in_maps, list(range(NC)), trace=trace, **(trace_kwargs or {})
    )
    out = assemble(res.results)
    if trace:
        _cached["last_result"] = res
    return out


if __name__ == "__main__":
    # smoke: random check against numpy reference
    rng = np.random.default_rng(0)
    x = rng.standard_normal((B, SEQ, DIM), dtype=np.float32)
    Wq = (rng.standard_normal((DIM, 1024), dtype=np.float32) * DIM ** -0.5)
    Wkv = (rng.standard_normal((DIM, 2048), dtype=np.float32) * DIM ** -0.5)
    pos_emb = rng.standard_normal((TABLE, D), dtype=np.float32)
    out = kernel(x, Wq, Wkv, pos_emb)
    print("out shape", out.shape, "finite:", np.isfinite(out).all())


# revision 60
# speedup vs baseline: 1.1778x; 1.1778x over previous
"""Trainium2 Bass kernel for relative-position attention (nn_Attention_14714557956326).

Full inputs:
  x       [4, 1024, 1024] f32
  Wq      [1024, 1024]    f32   (dim -> 16 heads * 64)
  Wkv     [1024, 2048]    f32   (k cols 0..1023, v cols 1024..2047)
  pos_emb [1025, 64]      f32
Output: [4, 16, 1024, 64] f32  (softmax(q k^T * s + rel_pos_bias) v, per head)

Sharding: 8 cores; core c handles batch c//2, heads 8*(c%2) .. +8.
Everything else is per-core local (no collectives).

v3 pipeline per (head h, row-tile t):
  ext  = q_t . Trev           (PE, psum; only the column window the skew reads)
  pos  = skew-DMA(ext)        (SP-issued diagonal DMA)
  S    = ID^T pos + q_t^T k   (PE, accumulated into one [128,1024] psum tile)
  p    = exp(S - C), z = sum  (Act, reads psum directly)
  p   *= 1/z                  (DVE)
  ptv  = transpose(p)         (DmaTranspose, issue alternating SP/Act)
  out_h= V^T ptv              (PE)
Head 0's ext/skew prep is emitted inside the projection phase so the
attention pipeline starts hot.
"""

import sys

sys.path.insert(0, "/opt/trn_rl_repo")

import numpy as np
from contextlib import ExitStack

import concourse.bass as bass
import concourse.bacc as bacc
import concourse.tile as tile
from concourse import mybir
from concourse.ap import AP
from concourse.bass_utils import run_bass_kernel_spmd

# ---------------- problem constants ----------------
B = 4
N_HEADS_TOT = 16
D = 64
DIM = 1024
SEQ = 1024
MAX_POS = 512
TABLE = 2 * MAX_POS + 1  # 1025
SCALE = D ** -0.5

NC = 8              # cores
NH = 8              # heads per core
HD = NH * D         # 512 projected cols per matrix per core
KC = DIM // 128     # 8 contraction chunks
IT = SEQ // 128     # 8 row tiles
RW = 1026           # on-chip table cols: cc=0..1023 = T[1024-cc]; cc=1024,1025 = T[0]
EXT = 2 * TABLE - 3  # 2047 ext width
C_SHIFT = 8.0       # exp(x - C) to keep fp16 P in range

F32 = mybir.dt.float32
F32R = mybir.dt.float32r
F16 = mybir.dt.float16
F8 = mybir.dt.float8e4
BF16 = mybir.dt.bfloat16

_cached = {}


def build_nc(seq=SEQ, nh=NH, bench_iters=1, ablate=(), G=4, pv_mode="v", skewq="sp", posdt="f16", trq="sp", indt="bf16"):
    ablate = set(ablate)
    """Build the per-core Bass program (SPMD: same program on all 8 cores)."""
    it = seq // 128
    hd = nh * D

    nc = bacc.Bacc(
        "TRN2",
        target_bir_lowering=False,
        debug=False,
        enable_asserts=False,
        num_devices=NC,
    )

    # DRAM inputs declared f32r/bf16 so DMA loads land directly in matmul-ready tiles
    IDT = F32R if indt == "f32" else BF16
    xT = nc.dram_tensor("xT", [DIM, seq], IDT, kind="ExternalInput")
    Wq = nc.dram_tensor("Wq", [DIM, hd], IDT, kind="ExternalInput")
    Wk = nc.dram_tensor("Wk", [DIM, hd], IDT, kind="ExternalInput")
    Wv = nc.dram_tensor("Wv", [DIM, hd], IDT, kind="ExternalInput")
    TrevT = nc.dram_tensor("TrevT", [D, RW], F32R, kind="ExternalInput")
    ID128 = nc.dram_tensor("ID128", [128, 128], F16, kind="ExternalInput")
    out_shape = [nh, seq, D] if pv_mode == "p" else [nh, D, seq]
    out = nc.dram_tensor("out", out_shape, F32, kind="ExternalOutput")

    with tile.TileContext(nc) as tc, ExitStack() as ctx:
        if bench_iters > 1:
            ctx.enter_context(
                tc.For_i(
                    0, bench_iters, 1,
                    hint_engines=(
                        mybir.EngineType.PE,
                        mybir.EngineType.DVE,
                        mybir.EngineType.Activation,
                        mybir.EngineType.SP,
                        mybir.EngineType.Pool,
                    ),
                    name="bench",
                )
            )
        # ---------------- persistent pools ----------------
        const_pool = ctx.enter_context(tc.tile_pool(name="const", bufs=1))
        qk_pool = ctx.enter_context(tc.tile_pool(name="qk", bufs=1))
        v_pool = ctx.enter_context(tc.tile_pool(name="v", bufs=1))
        # early attention pools: alive during projection (head-0 prep overlap)
        ext_pool = ctx.enter_context(tc.tile_pool(name="extp", bufs=1))
        pos_pool = ctx.enter_context(tc.tile_pool(name="posp", bufs=1))
        # PSUM pools (shared by projections and attention: 4+3+1 = 8 banks)
        psum_s = ctx.enter_context(tc.tile_pool(name="psS", bufs=2, space="PSUM"))
        psum_qp = ctx.enter_context(tc.tile_pool(name="psQP", bufs=3, space="PSUM"))
        psum_o = ctx.enter_context(tc.tile_pool(name="psO", bufs=1, space="PSUM"))

        # pos table, duplicated rows 0-63 and 64-127 so odd heads can slice base 64
        PDT = F16 if posdt == "f16" else F8
        trev_dup = const_pool.tile([128, RW], F32R, tag="trevdup")
        exp_bias = const_pool.tile([128, 1], F32, tag="expbias")
        id128_16 = const_pool.tile([128, 128], F16, tag="id128")
        nc.gpsimd.memset(exp_bias[:], -C_SHIFT)
        nc.gpsimd.dma_start(id128_16[:], ID128[:])
        if posdt == "f16":
            id128 = id128_16
        else:
            id128 = const_pool.tile([128, 128], F8, tag="id128f8")
            nc.vector.tensor_copy(id128[:], id128_16[:])
        nc.sync.dma_start(trev_dup[0:64, :], TrevT[:])
        nc.scalar.dma_start(trev_dup[64:128, :], TrevT[:])

        qT = [const_pool.tile([128, seq], F32R, name=f"qT{m}", tag=f"qT{m}") for m in range(nh // 2)]
        kT = [qk_pool.tile([128, seq], F32R, name=f"kT{m}", tag=f"kT{m}") for m in range(nh // 2)]
        V = [v_pool.tile([128, nh * D], F16, name=f"V{j}", tag=f"V{j}") for j in range(it)]

        poss = {}
        pts = {}

        # ext matmul column windows (trev cols cc in [A, B) are the only ones
        # the skew DMA for row-tile t can read; cc=1024 (T[0]) handled via VQ)
        def ext_window(t):
            base = seq - 1 - 128 * t
            lo = max(511, base - 127)
            hi = min(1535, base + 1023)
            A = (lo - 511) & ~3  # even width for fp32r matmul ISA rules
            return A, hi - 511 + 1  # trev col range [A, B); cc=1024 is T[0]

        def emit_qp_pair(hp, t, ci):
            if "noqp" in ablate:
                return
            # heads (2hp, 2hp+1) share one ext tile [128, 2*EXT] and ONE skew DMA
            q_pair = qT[hp]
            hs = (2 * hp, 2 * hp + 1)

            A, Bc = ext_window(t)
            ext = ext_pool.tile(
                [128, 2 * EXT], PDT, tag="ext", name=f"ext{hp}_{t}", bufs=4
            )
            lw = max(0, 511 - (seq - 128 - 128 * t))
            rw_ = min(max(0, (seq - 1 - 128 * t) + seq - 1 - 1535), EXT - 1536)
            if "noextcopy" in ablate:
                nc.gpsimd.memset(ext[:, 0:8], 0.01)
            for hh, h in enumerate(hs):
                base = 64 * hh
                q_t = q_pair[base : base + 64, 128 * t : 128 * (t + 1)]
                t_h = trev_dup[base : base + 64, :]
                eo = hh * EXT
                c0 = A
                qi = 0
                while c0 < Bc:
                    c1 = min(c0 + 512, Bc)
                    if (c1 - c0) % 2:
                        c1 += 1  # pad with the duplicate T[0] col (cc=1025)
                    ps_qp = psum_qp.tile(
                        [128, 512], F32, tag="QP", name=f"qp{h}_{t}_{qi}", bufs=3
                    )
                    nc.tensor.matmul(
                        ps_qp[:, : c1 - c0], q_t, t_h[:, c0:c1], start=True, stop=True
                    )
                    eng = (nc.vector, nc.vector, nc.scalar)[(2 * ci + 2 * hh + qi) % 3]
                    if "noextcopy" in ablate:
                        pass
                    elif eng is nc.vector:
                        eng.tensor_copy(ext[:, eo + 511 + c0 : eo + 511 + c1], ps_qp[:, : c1 - c0])
                    else:
                        eng.copy(ext[:, eo + 511 + c0 : eo + 511 + c1], ps_qp[:, : c1 - c0])
                    c0 = c1
                    qi += 1
                if "noextcopy" in ablate:
                    continue
                if lw > 0:
                    nc.gpsimd.tensor_copy(
                        ext[:, eo + 511 - lw : eo + 511],
                        ext[:, eo + 511 : eo + 512].to_broadcast([128, lw]),
                    )
                if rw_ > 0:
                    nc.gpsimd.tensor_copy(
                        ext[:, eo + 1536 : eo + 1536 + rw_],
                        ext[:, eo + 1535 : eo + 1536].to_broadcast([128, rw_]),
                    )

            exts[(hp, t)] = ext

        exts = {}

        def emit_skew_pair(hp, t):
            if "noqp" in ablate:
                return
            hs = (2 * hp, 2 * hp + 1)
            skip = "noskew" in ablate or ("halfskew" in ablate and t % 2 == 1)
            ext = exts.pop((hp, t))
            pos = pos_pool.tile(
                [128, 2 * seq], PDT, tag="pos", name=f"pos{hp}_{t}", bufs=12
            )
            poss[(hs[0], t)] = pos[:, 0:seq]
            poss[(hs[1], t)] = pos[:, seq : 2 * seq]
            extf = ext[:]
            diag = AP(
                tensor=extf.tensor,
                offset=extf.offset + (seq - 1 - 128 * t),
                ap=[[2 * EXT - 1, 128], [EXT, 2], [1, seq]],
            )
            if skip:
                nc.gpsimd.memset(pos[:, 0:8], 0.01)
                return
            if skewq == "sp":
                eng = nc.sync
            elif skewq == "sp+act":
                eng = (nc.sync, nc.scalar)[t % 2]
            elif skewq == "sp+pool":
                eng = (nc.sync, nc.gpsimd)[t % 2]
            elif skewq == "pool":
                eng = nc.gpsimd
            else:
                raise ValueError(skewq)
            eng.dma_start(pos[:], diag)

        # ---------------- projections (+ head-0 pos prep overlap) ----------------
        with tc.tile_pool(name="wres", bufs=2) as w_pool, \
             tc.tile_pool(name="xres", bufs=1) as x_pool:

            wcols = hd

            def load_w(wt, kc, ei):
                wr = w_pool.tile([128, wcols], IDT, tag=f"w{kc}")
                src = wt[128 * kc : 128 * (kc + 1), :]
                (nc.sync, nc.scalar, nc.gpsimd)[ei % 3].dma_start(wr[:], src)
                return wr

            # direct f32r loads (no staging copies); interleave x with wv so
            # the v-projection's per-kc operands arrive together
            x_r = []
            wv_chunks = []
            for kc in range(KC):
                xr = x_pool.tile([128, seq], IDT, tag=f"xr{kc}")
                (nc.sync, nc.scalar, nc.gpsimd)[kc % 3].dma_start(
                    xr[:], xT[128 * kc : 128 * (kc + 1), :]
                )
                x_r.append(xr)
                wv_chunks.append(load_w(Wv, kc, kc + 1))

            def emit_qkT(wname, w_chunks, m, ci):
                # one head-pair tile [128, seq] of q^T or k^T
                dest = qT[m] if wname == "q" else kT[m]
                for sh in range(seq // 512):
                    ps = psum_qp.tile([128, 512], F32, tag="QP", bufs=3)
                    for kc in range(KC):
                        nc.tensor.matmul(
                            ps[:],
                            w_chunks[kc][:, 128 * m : 128 * (m + 1)],
                            x_r[kc][:, 512 * sh : 512 * (sh + 1)],
                            start=(kc == 0),
                            stop=(kc == KC - 1),
                        )
                    dst = dest[:, 512 * sh : 512 * (sh + 1)]
                    if ci % 2 == 0:
                        nc.vector.tensor_copy(dst, ps[:])
                    else:
                        nc.scalar.copy(dst, ps[:])
                    ci += 1
                return ci

            # v: out [seq, hd] tiles -> V fp16; plus vq1024 [128, nh] (g=0 col)
            for jt in range(it):
                ps = psum_s.tile([128, seq], F32, tag="S", name=f"vp{jt}", bufs=2)
                for kc in range(KC):
                    nc.tensor.matmul(
                        ps[:, 0:hd],
                        x_r[kc][:, 128 * jt : 128 * (jt + 1)],
                        wv_chunks[kc][:, 0:hd],
                        start=(kc == 0),
                        stop=(kc == KC - 1),
                    )
                if jt % 2 == 0:
                    nc.vector.tensor_copy(V[jt][:], ps[:, 0:hd])
                else:
                    nc.scalar.copy(V[jt][:], ps[:, 0:hd])

            wq_chunks = [load_w(Wq, kc, kc + 1) for kc in range(KC)]
            ci = emit_qkT("q", wq_chunks, 0, 0)
            wk_chunks = [load_w(Wk, kc, kc) for kc in range(KC)]
            ci = emit_qkT("k", wk_chunks, 0, ci)

            # heads 0-1 pos prep overlaps the rest of projection
            cqp = 0
            for t in range(it):
                emit_qp_pair(0, t, cqp); cqp += 1
            for t in range(it):
                emit_skew_pair(0, t)
            for m in range(1, nh // 2):
                ci = emit_qkT("q", wq_chunks, m, ci)
            for m in range(1, nh // 2):
                ci = emit_qkT("k", wk_chunks, m, ci)

        # ---------------- attention ----------------
        att = ctx.enter_context(tc.tile_pool(name="att", bufs=3))
        pt_pool = ctx.enter_context(tc.tile_pool(name="pt", bufs=1))
        ot_pool = ctx.enter_context(tc.tile_pool(name="ot", bufs=3))

        ptalls, ptvs = {}, {}

        if trq == "sp":
            tr_engs = (nc.sync, nc.sync)
        elif trq == "act":
            tr_engs = (nc.scalar, nc.scalar)
        elif trq == "sp+act":
            tr_engs = (nc.sync, nc.scalar)
        else:
            raise ValueError(trq)

        def emit_tile(h, t):
            pair, base = h // 2, 64 * (h % 2)
            q_h = qT[pair][base : base + 64, :]
            k_h = kT[pair][base : base + 64, :]
            q_t = q_h[:, 128 * t : 128 * (t + 1)]
            pos = poss.pop((h, t), None)
            fake = "noqp" in ablate

            ps_s = psum_s.tile([128, seq], F32, tag="S", name=f"s{h}_{t}", bufs=2)
            for jh in range(seq // 512):
                sl = slice(512 * jh, 512 * (jh + 1))
                rhs_pos = V[t][:, 0:512] if fake else (pos[:, sl] if pos is not None else None)
                if "add" not in ablate and rhs_pos is not None:
                    nc.tensor.matmul(
                        ps_s[:, sl], id128[:], rhs_pos, start=True, stop=False
                    )
                    nc.tensor.matmul(ps_s[:, sl], q_t, k_h[:, sl], start=False, stop=True)
                else:
                    nc.tensor.matmul(ps_s[:, sl], q_t, k_h[:, sl], start=True, stop=True)

            if t % G == 0:
                pts["quad"] = att.tile(
                    [128, G * seq], F16, tag="pt", name=f"p{h}_{t}", bufs=8 // G + 2
                )
            p_pair = pts["quad"]
            p_t = p_pair[:, seq * (t % G) : seq * (t % G + 1)]
            z_t = att.tile([128, 1], F32, tag="zt", name=f"z{h}_{t}", bufs=10)
            if "exp" not in ablate:
                nc.scalar.activation(
                    p_t, ps_s[:], mybir.ActivationFunctionType.Exp,
                    bias=exp_bias[:], scale=1.0, accum_out=z_t[:],
                )
            else:
                nc.vector.tensor_copy(p_t, ps_s[:])
                nc.gpsimd.memset(z_t[:], 1.0)
            pts[(h, t)] = (p_pair, z_t)

        quads = {}

        def emit_norm(h, t, ti):
            p_quad, z_t = pts.pop((h, t))
            p_t = p_quad[:, seq * (t % G) : seq * (t % G + 1)]
            zr_t = att.tile([128, 1], F32, tag="zrt", name=f"zr{h}_{t}")
            nc.vector.reciprocal(zr_t[:], z_t[:])
            nc.vector.tensor_scalar_mul(p_t, p_t, zr_t[:])
            if t % G == G - 1:
                quads[(h, t // G)] = p_quad

        def emit_transpose(h, Q, ti):
            # one group transpose covers row-tiles G*Q..G*Q+G-1; blocks land
            # f-major: G*8 blocks of 128 at group offset G*1024*Q
            p_quad = quads.pop((h, Q))
            if G == 1:
                # baseline jb-major layout: x = jb*1024 + i
                dst = ptalls[h].rearrange("p (j i) -> p j i", j=it)[
                    :, :, 128 * Q : 128 * (Q + 1)
                ]
            else:
                dst = ptalls[h][:, G * seq * Q : G * seq * (Q + 1)].rearrange(
                    "p (b c) -> p b c", b=G * it
                )
            if "notr" in ablate:
                return
            teng = tr_engs[ti % 2]
            teng.dma_start_transpose(out=dst, in_=p_quad[:])

        ots = {}

        def emit_pv_part(h, pc):
            if "nopv" in ablate:
                return
            # i-chunk pair (2pc, 2pc+1); out rows i = 128*ch + p, layout [i, d].
            # ptall layout per group Q: x = G*1024*Q + qp*1024 + jb*128 + c
            # where i = G*128*Q + 128*qp + c, j = 128*jb + partition
            if G == 1:
                ptv3 = ptalls[h].rearrange("p (j i) -> p j i", j=it)
            else:
                ptv5 = ptalls[h].rearrange(
                    "p (Q qp jb c) -> p Q qp jb c", Q=it // G, qp=G, c=128
                )
            if pc == 0:
                oshape = [128, it * D] if pv_mode == "p" else [D, seq]
                ots[h] = ot_pool.tile(oshape, F32, tag="ot", name=f"ot{h}", bufs=3)
            ot = ots[h]
            def lhs_chunk(ch, jb):
                if G == 1:
                    return ptv3[:, jb, 128 * ch : 128 * (ch + 1)]
                return ptv5[:, ch // G, ch % G, jb, :]

            if pv_mode == "p":
                ps_o = psum_o.tile([128, 128], F32, tag="O", name=f"o{h}_{pc}")
                for ch in (2 * pc, 2 * pc + 1):
                    reg = ps_o[:, 64 * (ch % 2) : 64 * (ch % 2) + 64]
                    for jb in range(it):
                        nc.tensor.matmul(
                            reg,
                            lhs_chunk(ch, jb),
                            V[jb][:, D * h : D * (h + 1)],
                            start=(jb == 0),
                            stop=(jb == it - 1),
                        )
                nc.vector.tensor_copy(ot[:, 128 * pc : 128 * (pc + 1)], ps_o[:])
            else:
                # V-stationary: out [64 d, 256 i] per pc, moving = P^T chunks
                ps_o = psum_o.tile([64, 256], F32, tag="O", name=f"o{h}_{pc}")
                for ci2, ch in enumerate((2 * pc, 2 * pc + 1)):
                    reg = ps_o[:, 128 * ci2 : 128 * (ci2 + 1)]
                    for jb in range(it):
                        nc.tensor.matmul(
                            reg,
                            V[jb][:, D * h : D * (h + 1)],
                            lhs_chunk(ch, jb),
                            start=(jb == 0),
                            stop=(jb == it - 1),
                        )
                nc.vector.tensor_copy(ot[:, 256 * pc : 256 * (pc + 1)], ps_o[:])
            if pc == it // 2 - 1:
                if pv_mode == "p":
                    dstv = out[h].rearrange("(c p) d -> p c d", c=it)
                else:
                    dstv = out[h]
                nc.scalar.dma_start(dstv, ots.pop(h)[:])

        def emit_pv(h):
            for pc in range(it // 2):
                emit_pv_part(h, pc)

        cqp = 0
        ti = 0
        for h in range(nh):
            ptalls[h] = pt_pool.tile(
                [128, it * seq], F16, tag=f"ptall{h % 2}", name=f"ptall{h}"
            )
            if "notr" in ablate:
                nc.gpsimd.memset(ptalls[h][:, 0:8], 0.01)
        look = nh > 2
        for h in range(nh):
            for t in range(it):
                ahead = look and h % 2 == 0 and h + 2 < nh
                if ahead:
                    emit_qp_pair(h // 2 + 1, t, cqp); cqp += 1
                emit_tile(h, t)
                emit_norm(h, t, ti); ti += 1
                if ahead and t >= 2:
                    emit_skew_pair(h // 2 + 1, t - 2)
                if h > 0 and t % 2 == 0:
                    # transpose of the group pv(h-1) will need next tile
                    gq = t // G
                    if (h - 1, gq) in quads:
                        emit_transpose(h - 1, gq, ti)
                if h == nh - 1 and t % 2 == 0 and t // G - 1 >= 0:
                    gq = t // G - 1
                    if (h, gq) in quads:
                        emit_transpose(h, gq, ti)
                if h > 0 and t % 2 == 1:
                    emit_pv_part(h - 1, t // 2)
                if h == nh - 1 and t == 5:
                    # last head: drain first-half PV once groups 0-1 transposed
                    emit_pv_part(h, 0)
                    emit_pv_part(h, 1)
            if look and h % 2 == 0 and h + 2 < nh:
                emit_skew_pair(h // 2 + 1, it - 2)
                emit_skew_pair(h // 2 + 1, it - 1)
            if h == nh - 1:
                for Q in range(it // G):
                    if (h, Q) in quads:
                        emit_transpose(h, Q, ti)
        emit_pv_part(nh - 1, 2)
        emit_pv_part(nh - 1, 3)

    nc.compile()
    return nc


def prep_inputs(x, Wq, Wkv, pos_emb, indt="f32"):
    """Host-side shard prep: returns list of 8 per-core input dicts."""
    if indt == "bf16":
        import ml_dtypes
        cast = lambda a: np.asarray(a, dtype=ml_dtypes.bfloat16)
    else:
        cast = lambda a: a
    x = np.asarray(x, dtype=np.float32)
    Wq = np.asarray(Wq, dtype=np.float32)
    Wkv = np.asarray(Wkv, dtype=np.float32)
    pos_emb = np.asarray(pos_emb, dtype=np.float32)

    Wq_s = (Wq * SCALE).astype(np.float32)
    # [64, 1026]: col cc<=1023 -> T[1024-cc]; cols 1024,1025 -> T[0]
    trevT = np.ascontiguousarray(
        np.concatenate([pos_emb[::-1].T[:, :1024], pos_emb[0:1].T, pos_emb[0:1].T], axis=1)
    )
    id128 = np.eye(128, dtype=np.float16)

    in_maps = []
    for c in range(NC):
        b, hg = c // 2, c % 2
        hs = slice(HD * hg, HD * (hg + 1))
        wq_c = np.ascontiguousarray(Wq_s[:, hs])
        in_maps.append(
            {
                "xT": cast(np.ascontiguousarray(x[b].T)),
                "Wq": cast(wq_c),
                "Wk": cast(np.ascontiguousarray(Wkv[:, hs])),
                "Wv": cast(np.ascontiguousarray(Wkv[:, DIM:][:, hs])),
                "TrevT": trevT,
                "ID128": id128,
            }
        )
    return in_maps


def assemble(results):
    """results: list of 8 out maps -> [4,16,1024,64]."""
    out = np.empty((B, N_HEADS_TOT, SEQ, D), dtype=np.float32)
    for c in range(NC):
        b, hg = c // 2, c % 2
        r = results[c]["out"]
        if r.shape == (NH, D, SEQ):
            r = np.transpose(r, (0, 2, 1))
        out[b, NH * hg : NH * (hg + 1)] = r
    return out


def kernel(x, Wq, Wkv, pos_emb, trace=False, trace_kwargs=None, bench_iters=1, ablate=(), G=4, pv_mode="v", skewq="sp", posdt="f16", trq="sp", indt="bf16"):
    key = ("nc", bench_iters, tuple(sorted(ablate)), G, pv_mode, skewq, posdt, trq, indt)
    if key not in _cached:
        _cached[key] = build_nc(bench_iters=bench_iters, ablate=ablate, G=G, pv_mode=pv_mode, skewq=skewq, posdt=posdt, trq=trq, indt=indt)
    _cached["indt"] = indt
    nc = _cached[key]
    in_maps = prep_inputs(x, Wq, Wkv, pos_emb, indt=indt)
    res = run_bass_kernel_spmd(
        nc, in_maps, list(range(NC)), trace=trace, **(trace_kwargs or {})
    )
    out = assemble(res.results)
    if trace:
        _cached["last_result"] = res
    return out


if __name__ == "__main__":
    # smoke: random check against numpy reference
    rng = np.random.default_rng(0)
    x = rng.standard_normal((B, SEQ, DIM), dtype=np.float32)
    Wq = (rng.standard_normal((DIM, 1024), dtype=np.float32) * DIM ** -0.5)
    Wkv = (rng.standard_normal((DIM, 2048), dtype=np.float32) * DIM ** -0.5)
    pos_emb = rng.standard_normal((TABLE, D), dtype=np.float32)
    out = kernel(x, Wq, Wkv, pos_emb)
    print("out shape", out.shape, "finite:", np.isfinite(out).all())


# revision 63
# speedup vs baseline: 1.4479x; 1.2293x over previous
"""Trainium2 Bass kernel for relative-position attention (nn_Attention_14714557956326).

Full inputs:
  x       [4, 1024, 1024] f32
  Wq      [1024, 1024]    f32   (dim -> 16 heads * 64)
  Wkv     [1024, 2048]    f32   (k cols 0..1023, v cols 1024..2047)
  pos_emb [1025, 64]      f32
Output: [4, 16, 1024, 64] f32  (softmax(q k^T * s + rel_pos_bias) v, per head)

Sharding: 8 cores; core c handles batch c//2, heads 8*(c%2) .. +8.
Everything else is per-core local (no collectives).

v3 pipeline per (head h, row-tile t):
  ext  = q_t . Trev           (PE, psum; only the column window the skew reads)
  pos  = skew-DMA(ext)        (SP-issued diagonal DMA)
  S    = ID^T pos + q_t^T k   (PE, accumulated into one [128,1024] psum tile)
  p    = exp(S - C), z = sum  (Act, reads psum directly)
  p   *= 1/z                  (DVE)
  ptv  = transpose(p)         (DmaTranspose, issue alternating SP/Act)
  out_h= V^T ptv              (PE)
Head 0's ext/skew prep is emitted inside the projection phase so the
attention pipeline starts hot.
"""

import sys

sys.path.insert(0, "/opt/trn_rl_repo")

import numpy as np
from contextlib import ExitStack

import concourse.bass as bass
import concourse.bacc as bacc
import concourse.tile as tile
from concourse import mybir
from concourse.ap import AP
from concourse.bass_utils import run_bass_kernel_spmd

# ---------------- problem constants ----------------
B = 4
N_HEADS_TOT = 16
D = 64
DIM = 1024
SEQ = 1024
MAX_POS = 512
TABLE = 2 * MAX_POS + 1  # 1025
SCALE = D ** -0.5

NC = 8              # cores
NH = 8              # heads per core
HD = NH * D         # 512 projected cols per matrix per core
KC = DIM // 128     # 8 contraction chunks
IT = SEQ // 128     # 8 row tiles
RW = 1026           # on-chip table cols: cc=0..1023 = T[1024-cc]; cc=1024,1025 = T[0]
EXT = 2 * TABLE - 3  # 2047 ext width
C_SHIFT = 8.0       # exp(x - C) to keep fp16 P in range

F32 = mybir.dt.float32
F32R = mybir.dt.float32r
F16 = mybir.dt.float16
F8 = mybir.dt.float8e4
BF16 = mybir.dt.bfloat16

_cached = {}


def build_nc(seq=SEQ, nh=NH, bench_iters=1, ablate=(), G=4, pv_mode="v", skewq="sp", posdt="f16", trq="sp", indt="bf16", extq="dve", sdel=3, otq="dve", posb=12):
    ablate = set(ablate)
    """Build the per-core Bass program (SPMD: same program on all 8 cores)."""
    it = seq // 128
    hd = nh * D

    nc = bacc.Bacc(
        "TRN2",
        target_bir_lowering=False,
        debug=False,
        enable_asserts=False,
        num_devices=NC,
    )

    # DRAM inputs declared f32r/bf16 so DMA loads land directly in matmul-ready tiles
    IDT = F32R if indt == "f32" else BF16
    xT = nc.dram_tensor("xT", [DIM, seq], IDT, kind="ExternalInput")
    Wq = nc.dram_tensor("Wq", [DIM, hd], IDT, kind="ExternalInput")
    Wk = nc.dram_tensor("Wk", [DIM, hd], IDT, kind="ExternalInput")
    Wv = nc.dram_tensor("Wv", [DIM, hd], IDT, kind="ExternalInput")
    TrevT = nc.dram_tensor("TrevT", [D, RW], F32R, kind="ExternalInput")
    ID128 = nc.dram_tensor("ID128", [128, 128], F16, kind="ExternalInput")
    out_shape = [nh, seq, D] if pv_mode == "p" else [nh, D, seq]
    out = nc.dram_tensor("out", out_shape, F32, kind="ExternalOutput")

    with tile.TileContext(nc) as tc, ExitStack() as ctx:
        if bench_iters > 1:
            ctx.enter_context(
                tc.For_i(
                    0, bench_iters, 1,
                    hint_engines=(
                        mybir.EngineType.PE,
                        mybir.EngineType.DVE,
                        mybir.EngineType.Activation,
                        mybir.EngineType.SP,
                        mybir.EngineType.Pool,
                    ),
                    name="bench",
                )
            )
        # ---------------- persistent pools ----------------
        const_pool = ctx.enter_context(tc.tile_pool(name="const", bufs=1))
        qk_pool = ctx.enter_context(tc.tile_pool(name="qk", bufs=1))
        v_pool = ctx.enter_context(tc.tile_pool(name="v", bufs=1))
        # early attention pools: alive during projection (head-0 prep overlap)
        ext_pool = ctx.enter_context(tc.tile_pool(name="extp", bufs=1))
        pos_pool = ctx.enter_context(tc.tile_pool(name="posp", bufs=1))
        # PSUM pools (shared by projections and attention: 4+3+1 = 8 banks)
        psum_s = ctx.enter_context(tc.tile_pool(name="psS", bufs=2, space="PSUM"))
        psum_qp = ctx.enter_context(tc.tile_pool(name="psQP", bufs=3, space="PSUM"))
        psum_o = ctx.enter_context(tc.tile_pool(name="psO", bufs=1, space="PSUM"))

        # pos table, duplicated rows 0-63 and 64-127 so odd heads can slice base 64
        PDT = F16 if posdt == "f16" else F8
        trev_dup = const_pool.tile([128, RW], F32R, tag="trevdup")
        exp_bias = const_pool.tile([128, 1], F32, tag="expbias")
        id128_16 = const_pool.tile([128, 128], F16, tag="id128")
        nc.gpsimd.memset(exp_bias[:], -C_SHIFT)
        nc.gpsimd.dma_start(id128_16[:], ID128[:])
        if posdt == "f16":
            id128 = id128_16
        else:
            id128 = const_pool.tile([128, 128], F8, tag="id128f8")
            nc.vector.tensor_copy(id128[:], id128_16[:])
        nc.sync.dma_start(trev_dup[0:64, :], TrevT[:])
        nc.scalar.dma_start(trev_dup[64:128, :], TrevT[:])

        qT = [const_pool.tile([128, seq], F32R, name=f"qT{m}", tag=f"qT{m}") for m in range(nh // 2)]
        kT = [qk_pool.tile([128, seq], F32R, name=f"kT{m}", tag=f"kT{m}") for m in range(nh // 2)]
        V = [v_pool.tile([128, nh * D], F16, name=f"V{j}", tag=f"V{j}") for j in range(it)]

        poss = {}
        pts = {}

        # ext matmul column windows (trev cols cc in [A, B) are the only ones
        # the skew DMA for row-tile t can read; cc=1024 (T[0]) handled via VQ)
        def ext_window(t):
            base = seq - 1 - 128 * t
            lo = max(511, base - 127)
            hi = min(1535, base + 1023)
            A = (lo - 511) & ~3  # even width for fp32r matmul ISA rules
            return A, hi - 511 + 1  # trev col range [A, B); cc=1024 is T[0]

        def emit_qp_pair(hp, t, ci):
            if "noqp" in ablate:
                return
            # heads (2hp, 2hp+1) share one ext tile [128, 2*EXT] and ONE skew DMA
            q_pair = qT[hp]
            hs = (2 * hp, 2 * hp + 1)

            A, Bc = ext_window(t)
            ext = ext_pool.tile(
                [128, 2 * EXT], PDT, tag="ext", name=f"ext{hp}_{t}", bufs=4
            )
            lw = max(0, 511 - (seq - 128 - 128 * t))
            rw_ = min(max(0, (seq - 1 - 128 * t) + seq - 1 - 1535), EXT - 1536)
            if "noextcopy" in ablate:
                nc.gpsimd.memset(ext[:, 0:8], 0.01)
            for hh, h in enumerate(hs):
                base = 64 * hh
                q_t = q_pair[base : base + 64, 128 * t : 128 * (t + 1)]
                t_h = trev_dup[base : base + 64, :]
                eo = hh * EXT
                c0 = A
                qi = 0
                while c0 < Bc:
                    c1 = min(c0 + 512, Bc)
                    if (c1 - c0) % 2:
                        c1 += 1  # pad with the duplicate T[0] col (cc=1025)
                    ps_qp = psum_qp.tile(
                        [128, 512], F32, tag="QP", name=f"qp{h}_{t}_{qi}", bufs=3
                    )
                    nc.tensor.matmul(
                        ps_qp[:, : c1 - c0], q_t, t_h[:, c0:c1], start=True, stop=True
                    )
                    if extq == "dve":
                        eng = nc.vector
                    else:
                        eng = (nc.vector, nc.vector, nc.scalar)[(2 * ci + 2 * hh + qi) % 3]
                    if "noextcopy" in ablate:
                        pass
                    elif eng is nc.vector:
                        eng.tensor_copy(ext[:, eo + 511 + c0 : eo + 511 + c1], ps_qp[:, : c1 - c0])
                    else:
                        eng.copy(ext[:, eo + 511 + c0 : eo + 511 + c1], ps_qp[:, : c1 - c0])
                    c0 = c1
                    qi += 1
                if "noextcopy" in ablate:
                    continue
                if lw > 0:
                    nc.gpsimd.tensor_copy(
                        ext[:, eo + 511 - lw : eo + 511],
                        ext[:, eo + 511 : eo + 512].to_broadcast([128, lw]),
                    )
                if rw_ > 0:
                    nc.gpsimd.tensor_copy(
                        ext[:, eo + 1536 : eo + 1536 + rw_],
                        ext[:, eo + 1535 : eo + 1536].to_broadcast([128, rw_]),
                    )

            exts[(hp, t)] = ext

        exts = {}

        def emit_skew_pair(hp, t):
            if "noqp" in ablate:
                return
            hs = (2 * hp, 2 * hp + 1)
            skip = "noskew" in ablate or ("halfskew" in ablate and t % 2 == 1)
            ext = exts.pop((hp, t))
            pos = pos_pool.tile(
                [128, 2 * seq], PDT, tag="pos", name=f"pos{hp}_{t}", bufs=posb
            )
            poss[(hs[0], t)] = pos[:, 0:seq]
            poss[(hs[1], t)] = pos[:, seq : 2 * seq]
            extf = ext[:]
            diag = AP(
                tensor=extf.tensor,
                offset=extf.offset + (seq - 1 - 128 * t),
                ap=[[2 * EXT - 1, 128], [EXT, 2], [1, seq]],
            )
            if skip:
                nc.gpsimd.memset(pos[:, 0:8], 0.01)
                return
            if skewq == "sp":
                eng = nc.sync
            elif skewq == "sp+act":
                eng = (nc.sync, nc.scalar)[t % 2]
            elif skewq == "sp+pool":
                eng = (nc.sync, nc.gpsimd)[t % 2]
            elif skewq == "pool":
                eng = nc.gpsimd
            else:
                raise ValueError(skewq)
            eng.dma_start(pos[:], diag)

        # ---------------- projections (+ head-0 pos prep overlap) ----------------
        with tc.tile_pool(name="wres", bufs=2) as w_pool, \
             tc.tile_pool(name="xres", bufs=1) as x_pool:

            wcols = hd

            def load_w(wt, kc, ei):
                wr = w_pool.tile([128, wcols], IDT, tag=f"w{kc}")
                src = wt[128 * kc : 128 * (kc + 1), :]
                (nc.sync, nc.scalar, nc.gpsimd)[ei % 3].dma_start(wr[:], src)
                return wr

            # direct f32r loads (no staging copies); interleave x with wv so
            # the v-projection's per-kc operands arrive together
            x_r = []
            wv_chunks = []
            for kc in range(KC):
                xr = x_pool.tile([128, seq], IDT, tag=f"xr{kc}")
                (nc.sync, nc.scalar, nc.gpsimd)[kc % 3].dma_start(
                    xr[:], xT[128 * kc : 128 * (kc + 1), :]
                )
                x_r.append(xr)
                wv_chunks.append(load_w(Wv, kc, kc + 1))

            def emit_qkT(wname, w_chunks, m, ci):
                # one head-pair tile [128, seq] of q^T or k^T
                dest = qT[m] if wname == "q" else kT[m]
                for sh in range(seq // 512):
                    ps = psum_qp.tile([128, 512], F32, tag="QP", bufs=3)
                    for kc in range(KC):
                        nc.tensor.matmul(
                            ps[:],
                            w_chunks[kc][:, 128 * m : 128 * (m + 1)],
                            x_r[kc][:, 512 * sh : 512 * (sh + 1)],
                            start=(kc == 0),
                            stop=(kc == KC - 1),
                        )
                    dst = dest[:, 512 * sh : 512 * (sh + 1)]
                    if ci % 2 == 0:
                        nc.vector.tensor_copy(dst, ps[:])
                    else:
                        nc.scalar.copy(dst, ps[:])
                    ci += 1
                return ci

            # v: out [seq, hd] tiles -> V fp16; plus vq1024 [128, nh] (g=0 col)
            for jt in range(it):
                ps = psum_s.tile([128, seq], F32, tag="S", name=f"vp{jt}", bufs=2)
                for kc in range(KC):
                    nc.tensor.matmul(
                        ps[:, 0:hd],
                        x_r[kc][:, 128 * jt : 128 * (jt + 1)],
                        wv_chunks[kc][:, 0:hd],
                        start=(kc == 0),
                        stop=(kc == KC - 1),
                    )
                if jt % 2 == 0:
                    nc.vector.tensor_copy(V[jt][:], ps[:, 0:hd])
                else:
                    nc.scalar.copy(V[jt][:], ps[:, 0:hd])

            wq_chunks = [load_w(Wq, kc, kc + 1) for kc in range(KC)]
            ci = emit_qkT("q", wq_chunks, 0, 0)
            wk_chunks = [load_w(Wk, kc, kc) for kc in range(KC)]
            ci = emit_qkT("k", wk_chunks, 0, ci)

            # heads 0-1 pos prep overlaps the rest of projection
            cqp = 0
            for t in range(it):
                emit_qp_pair(0, t, cqp); cqp += 1
            for t in range(it):
                emit_skew_pair(0, t)
            for m in range(1, nh // 2):
                ci = emit_qkT("q", wq_chunks, m, ci)
            for m in range(1, nh // 2):
                ci = emit_qkT("k", wk_chunks, m, ci)

        # ---------------- attention ----------------
        att = ctx.enter_context(tc.tile_pool(name="att", bufs=3))
        pt_pool = ctx.enter_context(tc.tile_pool(name="pt", bufs=1))
        ot_pool = ctx.enter_context(tc.tile_pool(name="ot", bufs=3))

        ptalls, ptvs = {}, {}

        if trq == "sp":
            tr_engs = (nc.sync, nc.sync)
        elif trq == "act":
            tr_engs = (nc.scalar, nc.scalar)
        elif trq == "sp+act":
            tr_engs = (nc.sync, nc.scalar)
        else:
            raise ValueError(trq)

        def emit_tile(h, t):
            pair, base = h // 2, 64 * (h % 2)
            q_h = qT[pair][base : base + 64, :]
            k_h = kT[pair][base : base + 64, :]
            q_t = q_h[:, 128 * t : 128 * (t + 1)]
            pos = poss.pop((h, t), None)
            fake = "noqp" in ablate

            ps_s = psum_s.tile([128, seq], F32, tag="S", name=f"s{h}_{t}", bufs=2)
            for jh in range(seq // 512):
                sl = slice(512 * jh, 512 * (jh + 1))
                rhs_pos = V[t][:, 0:512] if fake else (pos[:, sl] if pos is not None else None)
                if "add" not in ablate and rhs_pos is not None:
                    nc.tensor.matmul(
                        ps_s[:, sl], id128[:], rhs_pos, start=True, stop=False
                    )
                    nc.tensor.matmul(ps_s[:, sl], q_t, k_h[:, sl], start=False, stop=True)
                else:
                    nc.tensor.matmul(ps_s[:, sl], q_t, k_h[:, sl], start=True, stop=True)

            if t % G == 0:
                pts["quad"] = att.tile(
                    [128, G * seq], F16, tag="pt", name=f"p{h}_{t}", bufs=8 // G + 2
                )
            p_pair = pts["quad"]
            p_t = p_pair[:, seq * (t % G) : seq * (t % G + 1)]
            z_t = att.tile([128, 1], F32, tag="zt", name=f"z{h}_{t}", bufs=10)
            if "exp" not in ablate:
                nc.scalar.activation(
                    p_t, ps_s[:], mybir.ActivationFunctionType.Exp,
                    bias=exp_bias[:], scale=1.0, accum_out=z_t[:],
                )
            else:
                nc.vector.tensor_copy(p_t, ps_s[:])
                nc.gpsimd.memset(z_t[:], 1.0)
            pts[(h, t)] = (p_pair, z_t)

        quads = {}

        def emit_norm(h, t, ti):
            p_quad, z_t = pts.pop((h, t))
            p_t = p_quad[:, seq * (t % G) : seq * (t % G + 1)]
            zr_t = att.tile([128, 1], F32, tag="zrt", name=f"zr{h}_{t}")
            nc.vector.reciprocal(zr_t[:], z_t[:])
            nc.vector.tensor_scalar_mul(p_t, p_t, zr_t[:])
            if t % G == G - 1:
                quads[(h, t // G)] = p_quad

        def emit_transpose(h, Q, ti):
            # one group transpose covers row-tiles G*Q..G*Q+G-1; blocks land
            # f-major: G*8 blocks of 128 at group offset G*1024*Q
            p_quad = quads.pop((h, Q))
            if G == 1:
                # baseline jb-major layout: x = jb*1024 + i
                dst = ptalls[h].rearrange("p (j i) -> p j i", j=it)[
                    :, :, 128 * Q : 128 * (Q + 1)
                ]
            else:
                dst = ptalls[h][:, G * seq * Q : G * seq * (Q + 1)].rearrange(
                    "p (b c) -> p b c", b=G * it
                )
            if "notr" in ablate:
                return
            teng = tr_engs[ti % 2]
            teng.dma_start_transpose(out=dst, in_=p_quad[:])

        ots = {}

        def emit_pv_part(h, pc):
            if "nopv" in ablate:
                return
            # i-chunk pair (2pc, 2pc+1); out rows i = 128*ch + p, layout [i, d].
            # ptall layout per group Q: x = G*1024*Q + qp*1024 + jb*128 + c
            # where i = G*128*Q + 128*qp + c, j = 128*jb + partition
            if G == 1:
                ptv3 = ptalls[h].rearrange("p (j i) -> p j i", j=it)
            else:
                ptv5 = ptalls[h].rearrange(
                    "p (Q qp jb c) -> p Q qp jb c", Q=it // G, qp=G, c=128
                )
            if pc == 0:
                oshape = [128, it * D] if pv_mode == "p" else [D, seq]
                ots[h] = ot_pool.tile(oshape, F32, tag="ot", name=f"ot{h}", bufs=3)
            ot = ots[h]
            def lhs_chunk(ch, jb):
                if G == 1:
                    return ptv3[:, jb, 128 * ch : 128 * (ch + 1)]
                return ptv5[:, ch // G, ch % G, jb, :]

            if pv_mode == "p":
                ps_o = psum_o.tile([128, 128], F32, tag="O", name=f"o{h}_{pc}")
                for ch in (2 * pc, 2 * pc + 1):
                    reg = ps_o[:, 64 * (ch % 2) : 64 * (ch % 2) + 64]
                    for jb in range(it):
                        nc.tensor.matmul(
                            reg,
                            lhs_chunk(ch, jb),
                            V[jb][:, D * h : D * (h + 1)],
                            start=(jb == 0),
                            stop=(jb == it - 1),
                        )
                nc.vector.tensor_copy(ot[:, 128 * pc : 128 * (pc + 1)], ps_o[:])
            else:
                # V-stationary: out [64 d, 256 i] per pc, moving = P^T chunks
                ps_o = psum_o.tile([64, 256], F32, tag="O", name=f"o{h}_{pc}")
                for ci2, ch in enumerate((2 * pc, 2 * pc + 1)):
                    reg = ps_o[:, 128 * ci2 : 128 * (ci2 + 1)]
                    for jb in range(it):
                        nc.tensor.matmul(
                            reg,
                            V[jb][:, D * h : D * (h + 1)],
                            lhs_chunk(ch, jb),
                            start=(jb == 0),
                            stop=(jb == it - 1),
                        )
                if otq == "dve":
                    nc.vector.tensor_copy(ot[:, 256 * pc : 256 * (pc + 1)], ps_o[:])
                else:
                    nc.scalar.copy(ot[:, 256 * pc : 256 * (pc + 1)], ps_o[:])
            if pc == it // 2 - 1:
                if pv_mode == "p":
                    dstv = out[h].rearrange("(c p) d -> p c d", c=it)
                else:
                    dstv = out[h]
                nc.scalar.dma_start(dstv, ots.pop(h)[:])

        def emit_pv(h):
            for pc in range(it // 2):
                emit_pv_part(h, pc)

        cqp = 0
        ti = 0
        for h in range(nh):
            ptalls[h] = pt_pool.tile(
                [128, it * seq], F16, tag=f"ptall{h % 2}", name=f"ptall{h}"
            )
            if "notr" in ablate:
                nc.gpsimd.memset(ptalls[h][:, 0:8], 0.01)
        look = nh > 2
        for h in range(nh):
            for t in range(it):
                ahead = look and h % 2 == 0 and h + 2 < nh
                if ahead:
                    emit_qp_pair(h // 2 + 1, t, cqp); cqp += 1
                emit_tile(h, t)
                emit_norm(h, t, ti); ti += 1
                if ahead and t >= sdel:
                    emit_skew_pair(h // 2 + 1, t - sdel)
                if h > 0 and t % 2 == 0:
                    # transpose of the group pv(h-1) will need next tile
                    gq = t // G
                    if (h - 1, gq) in quads:
                        emit_transpose(h - 1, gq, ti)
                if h == nh - 1 and t % 2 == 0 and t // G - 1 >= 0:
                    gq = t // G - 1
                    if (h, gq) in quads:
                        emit_transpose(h, gq, ti)
                if h > 0 and t % 2 == 1:
                    emit_pv_part(h - 1, t // 2)
                if h == nh - 1 and t == 5:
                    # last head: drain first-half PV once groups 0-1 transposed
                    emit_pv_part(h, 0)
                    emit_pv_part(h, 1)
            if look and h % 2 == 0 and h + 2 < nh:
                for tl in range(it - sdel, it):
                    emit_skew_pair(h // 2 + 1, tl)
            if h == nh - 1:
                for Q in range(it // G):
                    if (h, Q) in quads:
                        emit_transpose(h, Q, ti)
        emit_pv_part(nh - 1, 2)
        emit_pv_part(nh - 1, 3)

    nc.compile()
    return nc


def prep_inputs(x, Wq, Wkv, pos_emb, indt="f32"):
    """Host-side shard prep: returns list of 8 per-core input dicts."""
    if indt == "bf16":
        import ml_dtypes
        cast = lambda a: np.asarray(a, dtype=ml_dtypes.bfloat16)
    else:
        cast = lambda a: a
    x = np.asarray(x, dtype=np.float32)
    Wq = np.asarray(Wq, dtype=np.float32)
    Wkv = np.asarray(Wkv, dtype=np.float32)
    pos_emb = np.asarray(pos_emb, dtype=np.float32)

    Wq_s = (Wq * SCALE).astype(np.float32)
    # [64, 1026]: col cc<=1023 -> T[1024-cc]; cols 1024,1025 -> T[0]
    trevT = np.ascontiguousarray(
        np.concatenate([pos_emb[::-1].T[:, :1024], pos_emb[0:1].T, pos_emb[0:1].T], axis=1)
    )
    id128 = np.eye(128, dtype=np.float16)

    in_maps = []
    for c in range(NC):
        b, hg = c // 2, c % 2
        hs = slice(HD * hg, HD * (hg + 1))
        wq_c = np.ascontiguousarray(Wq_s[:, hs])
        in_maps.append(
            {
                "xT": cast(np.ascontiguousarray(x[b].T)),
                "Wq": cast(wq_c),
                "Wk": cast(np.ascontiguousarray(Wkv[:, hs])),
                "Wv": cast(np.ascontiguousarray(Wkv[:, DIM:][:, hs])),
                "TrevT": trevT,
                "ID128": id128,
            }
        )
    return in_maps


def assemble(results):
    """results: list of 8 out maps -> [4,16,1024,64]."""
    out = np.empty((B, N_HEADS_TOT, SEQ, D), dtype=np.float32)
    for c in range(NC):
        b, hg = c // 2, c % 2
        r = results[c]["out"]
        if r.shape == (NH, D, SEQ):
            r = np.transpose(r, (0, 2, 1))
        out[b, NH * hg : NH * (hg + 1)] = r
    return out


def kernel(x, Wq, Wkv, pos_emb, trace=False, trace_kwargs=None, bench_iters=1, ablate=(), G=4, pv_mode="v", skewq="sp", posdt="f16", trq="sp", indt="bf16", extq="dve", sdel=3, otq="dve", posb=12):
    if indt == "bf16":
        try:
            import ml_dtypes  # noqa: F401
        except ImportError:
            indt = "f32"
    key = ("nc", bench_iters, tuple(sorted(ablate)), G, pv_mode, skewq, posdt, trq, indt, extq, sdel, otq, posb)
    if key not in _cached:
        _cached[key] = build_nc(bench_iters=bench_iters, ablate=ablate, G=G, pv_mode=pv_mode, skewq=skewq, posdt=posdt, trq=trq, indt=indt, extq=extq, sdel=sdel, otq=otq, posb=posb)
    nc = _cached[key]
    in_maps = prep_inputs(x, Wq, Wkv, pos_emb, indt=indt)
    res = run_bass_kernel_spmd(
        nc, in_maps, list(range(NC)), trace=trace, **(trace_kwargs or {})
    )
    out = assemble(res.results)
    if trace:
        _cached["last_result"] = res
    return out


if __name__ == "__main__":
    # smoke: random check against numpy reference
    rng = np.random.default_rng(0)
    x = rng.standard_normal((B, SEQ, DIM), dtype=np.float32)
    Wq = (rng.standard_normal((DIM, 1024), dtype=np.float32) * DIM ** -0.5)
    Wkv = (rng.standard_normal((DIM, 2048), dtype=np.float32) * DIM ** -0.5)
    pos_emb = rng.standard_normal((TABLE, D), dtype=np.float32)
    out = kernel(x, Wq, Wkv, pos_emb)
    print("out shape", out.shape, "finite:", np.isfinite(out).all())
